# revision 45
# baseline (speedup 1.0000x reference)
"""Trainium2 Bass kernel for the int8-fake-quant double-conv model.

Math: all fake-quantized values are integers times power-of-2 scales, so every
intermediate is exactly representable in bf16 (|int| <= 128) and every conv
accumulation is exact in fp32 PSUM. Conv1 is mapped onto the 128x128 PE
array with banded-Toeplitz stationary matrices; the 3 horizontal taps are 3
PSUM-accumulated matmuls with the rhs shifted along the free dim. Per-tensor
bias rides as an extra K row against a persistent ones-row. Rounding to the
quant grid is the fp32 magic-number trick (+/- 1.5*2^23*scale), which is RNE
and matches jnp.round exactly; the int8 clamps are provably inactive for this
input distribution (|x|<5.5, |y|<4.5, |z/s|<77 vs bound 128), so quantization
is a single fused tensor_scalar per tensor.

Conv2 runs TRANSPOSED ("T-form"): the yq half block [121=(10ci x 12row +
ones), w] is the STATIONARY operand, sliced per 128-wide width tile with a
+dj column shift, and the banded-Toeplitz weight matrix that the classic
orientation used as stationary is the MOVING operand [121, 100=(10co x
10ol)]. PSUM comes out width-major [128 wm, (wt,co,ol)]. This hits the
dataflow floor of 3 writes per z element / 128-wide M: conv2 costs 12
matmuls x N=100 = 1200 PE rows per 10-row z group vs 3 x 508 = 1524 for the
classic orientation (PE busy/super 2.34us vs 2.54us). The weight matrices
are identical either way -- only the operand roles swap.

Per-core layout (4 images, data-parallel over batch):
  A "super" covers a pair of 10-row z blocks: 24 x rows are loaded with ONE
  tall DMA as [120=(5ci x 24row), 512] and quantized with ONE gpsimd op.
  Conv1 uses per-half stationaries over the shared 24-row window (all
  stationaries live in one packed [121,1560] const, one DMA), producing two
  overlapping 12-row y blocks in per-half PSUM tiles [120,512]. A fused DVE
  tensor_scalar per half quantizes y right after its half's matmuls (low
  latency into conv2). Conv2-T accumulates the 3 dj taps per width tile
  into a [128,400] PSUM strip per half; the z epilogue is split so no
  engine saturates and PSUM frees fast: h0 is one fused DVE tensor_scalar
  straight from PSUM (keeps the Act dependency off the DVE stream, which
  must stay clear for y-quants), h1 goes Act(+magic) -> Pool(-magic bf16).
  One store DMA per super: out is
  laid out [b,super,128,800=(h,wt,co,ol)]; the host transposes back,
  unpads, and upcasts (bf16->fp32 is exact). Width tile wt=3 columns
  beyond z col 507 accumulate stale-SBUF garbage (confined to wm>=124
  there by the AP geometry); the host drops them. The PE stream is forced
  via ordering-only deps into the gap-free interleave
  c1(s)h0, c1(s)h1, c2(s-1)h1, c2(s)h0 -- the scheduler's greedy order
  otherwise serializes on the y-quant latency.

Engines/super: PE 2.34us (97% busy, bound); DVE ~1.9; Act ~0.5; Pool ~1.5;
DMA ~1.25. Sim/HW exec: 242168 ns (prev best 270735, stub 394615).
"""

from contextlib import nullcontext as _nullctx

import numpy as np
import ml_dtypes

import concourse.bacc as bacc
import concourse.bass as bass
import concourse.mybir as mybir
import concourse.tile as tile
from concourse.tile import add_dep_helper
from concourse import bass_utils

BF16 = ml_dtypes.bfloat16
N_CORES = 8
B_PER_CORE = 4
H = W = 512
H1, W1 = 510, 510      # conv1 output
H2, W2 = 508, 508      # conv2 output
CIN, CMID, COUT = 5, 10, 10
SUPERS_PER_B = 26      # 25 main supers (20 z rows each) + 1 edge (8 z rows)
WP = 516               # per-half column span in the yq ring tiles
NWT = 4                # 128-wide conv2-T width tiles per z group

_prog_cache = {}


def _fq(a, s):
    return (np.clip(np.rint(np.asarray(a, np.float32) / np.float32(s)),
                    -128, 127) * np.float32(s)).astype(np.float32)


def _make_consts(w1, b1, w2, b2, s_in, s_w1, s_o1, s_w2, s_o2):
    s_in, s_w1, s_o1, s_w2, s_o2 = (float(np.asarray(v).reshape(-1)[0])
                                    for v in (s_in, s_w1, s_o1, s_w2, s_o2))
    for s in (s_in, s_w1, s_o1, s_w2, s_o2):
        m, e = np.frexp(np.float64(s))
        assert m == 0.5, f"scale {s} not a power of two; exact path invalid"

    w1q = _fq(w1, s_w1)
    b1q = _fq(b1, s_in * s_w1)
    w2q = _fq(w2, s_w2)
    b2q = _fq(b2, s_o1 * s_w2)

    c = {}
    # conv1 main: K = (ci,j) over the 24-row x window (ci*24+j), plus bias row
    # at 120 (dj==0 only). M = (co,il): co*12+il. Half h computes y rows
    # (20s+10h)+il from x rows j = 10h+il+di.
    for h in range(2):
        for dj in range(3):
            S = np.zeros((121, 120), np.float32)
            for ci in range(CIN):
                for il in range(12):
                    for di in range(3):
                        j = 10 * h + il + di
                        S[ci * 24 + j, il::12] = w1q[:, ci, di, dj]
            if dj == 0:
                S[120, :] = np.repeat(b1q, 12)
            c[f"s1_{h}_{dj}"] = S
    # conv1 edge (10 y rows from 12 x rows): K = ci*12+j, bias row at 120.
    # M = co*10+il.
    for dj in range(3):
        S = np.zeros((121, 100), np.float32)
        for ci in range(CIN):
            for il in range(10):
                for di in range(3):
                    S[ci * 12 + il + di, il::10] = w1q[:, ci, di, dj]
        if dj == 0:
            S[120, :] = np.repeat(b1q, 10)
        c[f"s1e_{dj}"] = S
    # conv2 main: K = (ci,iw) block-local (ci*12+iw), bias row 120.
    # M = co*10+ol; z row ol uses y rows iw = ol+di. Same S for both halves.
    for dj in range(3):
        S = np.zeros((121, 100), np.float32)
        for ci in range(CMID):
            for ol in range(10):
                for di in range(3):
                    S[ci * 12 + ol + di, ol::10] = w2q[:, ci, di, dj]
        if dj == 0:
            S[120, :] = np.repeat(b2q, 10)
        c[f"s2_{dj}"] = S
    # conv2 edge: K = ci*10+iw (edge y block is 10 rows), M = co*8+ol.
    for dj in range(3):
        S = np.zeros((121, 80), np.float32)
        for ci in range(CMID):
            for ol in range(8):
                for di in range(3):
                    S[ci * 10 + ol + di, ol::8] = w2q[:, ci, di, dj]
        if dj == 0:
            S[120, :] = np.repeat(b2q, 8)
        c[f"s2e_{dj}"] = S

    consts = {k: v.astype(BF16) for k, v in c.items()}
    for k, v in c.items():
        assert np.array_equal(consts[k].astype(np.float32), v), \
            f"bf16 cast lossy for {k}; exact path invalid"
    scal = {"m4x": np.float32(1.5 * 2**23 * s_in),
            "m4y": np.float32(1.5 * 2**23 * s_o1),
            "m4z": np.float32(1.5 * 2**23 * s_o2)}
    return consts, scal


S_SHAPES = {**{f"s1_{h}_{dj}": (121, 120) for h in range(2) for dj in range(3)},
            **{f"s1e_{dj}": (121, 100) for dj in range(3)},
            **{f"s2_{dj}": (121, 100) for dj in range(3)},
            **{f"s2e_{dj}": (121, 80) for dj in range(3)}}

# packed stationary layout: one [121, S_TOT] tensor, one DMA
S_OFF = {}
_off = 0
for _k in S_SHAPES:
    S_OFF[_k] = _off
    _off += S_SHAPES[_k][1]
S_TOT = _off


def pack_smat(consts):
    smat = np.zeros((121, S_TOT), dtype=BF16)
    for k, v in consts.items():
        smat[:, S_OFF[k]:S_OFF[k] + v.shape[1]] = v
    return smat


def build_program(scal, D1=4, D2=3, XRB=3, XQR=7, YQR=7, ZOB=4, P1B=2, P2B=2,
                  YTB=3, ZPRI=0, WNUM=4, WN=512, QDV=1, TFUSE=0,
                  SWP0=0, SWPE=0, SMAT2_IT=0):
    """D1: supers between x load/quant and conv1; D2: between conv1 and
    conv2."""
    nc = bacc.Bacc("TRN2", target_bir_lowering=False, debug=False,
                   num_devices=N_CORES)
    f32, bf = mybir.dt.float32, mybir.dt.bfloat16
    ADD, SUB = mybir.AluOpType.add, mybir.AluOpType.subtract
    COPY = mybir.ActivationFunctionType.Copy
    m4x, m4y, m4z = (float(scal["m4x"]), float(scal["m4y"]), float(scal["m4z"]))

    x_d = nc.dram_tensor("x", [B_PER_CORE, CIN, H, W], f32, kind="ExternalInput")
    # out layout [b, super, wm, (h,wt,co,ol)]: one plain-slice store DMA per
    # super matches the width-major zo layout; host reorders on unpack
    out_d = nc.dram_tensor("out", [B_PER_CORE, SUPERS_PER_B, 128, 800],
                           bf, kind="ExternalOutput")
    smat_d = nc.dram_tensor("smat", [121, S_TOT], bf, kind="ExternalInput")

    with tile.TileContext(nc) as tc:
        with (tc.tile_pool(name="consts", bufs=1) as cpool,
              tc.tile_pool(name="xraw", bufs=XRB) as xraw_pool,
              tc.tile_pool(name="xq", bufs=1) as xq_pool,
              tc.tile_pool(name="yq", bufs=1) as yq_pool,
              tc.tile_pool(name="ytmp", bufs=YTB) as ytmp_pool,
              tc.tile_pool(name="zout", bufs=ZOB) as zout_pool,
              tc.tile_pool(name="p1", bufs=P1B, space=bass.MemorySpace.PSUM) as p1_pool,
              tc.tile_pool(name="p2", bufs=P2B, space=bass.MemorySpace.PSUM) as p2_pool):

            smat_t = cpool.tile([121, S_TOT], bf, tag="smat", name="smat")

            def s_t(k, kx):
                off = S_OFF[k]
                return smat_t[0:kx, off:off + S_SHAPES[k][1]]

            # ring tiles with persistent ones-row at partition 120
            xq_ring = [xq_pool.tile([121, W], bf, tag=f"xq{i}", name=f"xq{i}")
                       for i in range(XQR)]
            # conv2-T stationary slices read up to col 516h+513 (wt=3, dj=2),
            # so each half block gets a 516-col span
            yq_ring = [yq_pool.tile([121, 2 * WP], bf, tag=f"yq{i}",
                                    name=f"yq{i}") for i in range(YQR)]
            # ones-row init is interleaved into each ring tile's first use
            # (engine partition starts must be 32-aligned: memset [96:121];
            # rows 96..119 are data rows that every quant op rewrites)

            def load(sit):
                b, s = divmod(sit, SUPERS_PER_B)
                edge = (s == SUPERS_PER_B - 1)
                xq_t = xq_ring[sit % XQR]
                if sit < XQR:
                    eng = nc.vector if sit % 2 else nc.gpsimd
                    eng.memset(xq_t[96:121, :], 1.0)
                xr = xraw_pool.tile([120, W], f32, tag="xr", name="xr")
                if edge:
                    nc.sync.dma_start(xr[0:60, :], x_d.ap()[b, :, 500:512, :])
                    q_ops[sit] = nc.gpsimd.tensor_scalar(
                        xq_t[0:60, :], xr[0:60, :], m4x, m4x, ADD, SUB)
                else:
                    r0 = 20 * s
                    nc.sync.dma_start(xr[0:120, :],
                                      x_d.ap()[b, :, r0:r0 + 24, :])
                    # first supers: quant on the still-idle DVE (faster than
                    # Pool, which is busy with ring-fill memsets) so the
                    # startup conv1s aren't gated on Pool
                    qeng = nc.vector if sit <= QDV else nc.gpsimd
                    q_ops[sit] = qeng.tensor_scalar(
                        xq_t[0:120, :], xr[0:120, :], m4x, m4x, ADD, SUB)

            def conv1(sit):
                b, s = divmod(sit, SUPERS_PER_B)
                edge = (s == SUPERS_PER_B - 1)
                xq_t = xq_ring[sit % XQR]
                yq_t = yq_ring[sit % YQR]
                if sit < YQR:
                    eng = nc.vector if sit % 2 else nc.gpsimd
                    ms = eng.memset(yq_t[96:121, :], 1.0)
                    # keep fill-time memsets from beating critical x-quants
                    # to an idle engine (ordering-only)
                    if sit >= 3 and (sit - 3) in q_ops:
                        add_dep_helper(ms.ins, q_ops[sit - 3].ins, sync=False,
                                       reason="memset after quant")
                g1[sit] = []
                if edge:
                    p1 = p1_pool.tile([120, 512], f32, tag="p1a", name="p1a")
                    for dj in range(3):
                        kx = 121 if dj == 0 else 120
                        mm = nc.tensor.matmul(p1[0:100, 0:W1],
                                              s_t(f"s1e_{dj}", kx),
                                              xq_t[0:kx, dj:dj + W1],
                                              start=(dj == 0), stop=(dj == 2))
                        if dj == 0:
                            first = mm
                    g1[sit].append((first, mm))
                    nc.vector.tensor_scalar(yq_t[0:100, 0:W], p1[0:100, 0:W],
                                            m4y, m4y, ADD, SUB)
                else:
                    # separate PSUM tile per half (bank-granular independence)
                    # and per-half y-quant right after each half's matmuls:
                    # keeps the yq RAW latency off the PE critical path
                    for h in range(2):
                        p1 = p1_pool.tile([120, 512], f32, tag=f"p1{'ab'[h]}",
                                          name=f"p1{'ab'[h]}")
                        for dj in range(3):
                            kx = 121 if dj == 0 else 120
                            mm = nc.tensor.matmul(p1[0:120, 0:W1],
                                                  s_t(f"s1_{h}_{dj}", kx),
                                                  xq_t[0:kx, dj:dj + W1],
                                                  start=(dj == 0), stop=(dj == 2))
                            if dj == 0:
                                first = mm
                        g1[sit].append((first, mm))
                        nc.vector.tensor_scalar(
                            yq_t[0:120, WP * h:WP * h + W],
                            p1[0:120, 0:W],
                            m4y, m4y, ADD, SUB)

            def conv2(sit):
                # conv2-T: yq half block is the STATIONARY (sliced per width
                # tile + dj shift), the Toeplitz weight matrix is the MOVING
                # operand; z lands width-major [128 wm, (wt,co,ol)]
                b, s = divmod(sit, SUPERS_PER_B)
                edge = (s == SUPERS_PER_B - 1)
                yq_t = yq_ring[sit % YQR]
                zo = zout_pool.tile([128, 800], bf, tag="zo", name="zo")
                g2[sit] = []
                if edge:
                    p2 = p2_pool.tile([128, 400], f32, tag="p2a", name="p2a")
                    mms = []
                    for wt in range(NWT):
                        for dj in range(3):
                            ky = 121 if dj == 0 else 120
                            c0 = 128 * wt + dj
                            mms.append(nc.tensor.matmul(
                                p2[0:128, 80 * wt:80 * wt + 80],
                                yq_t[0:ky, c0:c0 + 128],
                                s_t(f"s2e_{dj}", ky),
                                start=(dj == 0), stop=(dj == 2)))
                    for pm, nm in zip(mms, mms[1:]):
                        add_dep_helper(nm.ins, pm.ins, sync=False,
                                       reason="c2T order")
                    g2[sit].append((mms[0], mms[-1]))
                    nc.vector.tensor_scalar(zo[0:128, 0:320], p2[0:128, 0:320],
                                            m4z, m4z, ADD, SUB)
                    nc.sync.dma_start(
                        out_d.ap()[b, SUPERS_PER_B - 1, :, 0:320],
                        zo[0:128, 0:320])
                else:
                    p2s = []
                    for h in range(2):
                        p2 = p2_pool.tile([128, 400], f32, tag=f"p2{'ab'[h]}",
                                          name=f"p2{'ab'[h]}")
                        p2s.append(p2)
                        mms = []
                        for wt in range(NWT):
                            for dj in range(3):
                                ky = 121 if dj == 0 else 120
                                c0 = WP * h + 128 * wt + dj
                                mms.append(nc.tensor.matmul(
                                    p2[0:128, 100 * wt:100 * wt + 100],
                                    yq_t[0:ky, c0:c0 + 128],
                                    s_t(f"s2_{dj}", ky),
                                    start=(dj == 0), stop=(dj == 2)))
                        for pm, nm in zip(mms, mms[1:]):
                            add_dep_helper(nm.ins, pm.ins, sync=False,
                                           reason="c2T order")
                        g2[sit].append((mms[0], mms[-1]))
                    with tc.high_priority(offset=ZPRI) if ZPRI else _nullctx():
                        # h0: single fused quant on DVE straight from PSUM
                        # (no Act dependency); h1: Act +magic then Pool -magic
                        nc.vector.tensor_scalar(zo[0:128, 0:400],
                                                p2s[0][0:128, 0:400],
                                                m4z, m4z, ADD, SUB)
                        if sit >= B_PER_CORE * SUPERS_PER_B - 2 - TFUSE:
                            # last main super: fuse h1 on DVE too so its
                            # store clears HWDGE before the edge's final DMA
                            nc.vector.tensor_scalar(zo[0:128, 400:800],
                                                    p2s[1][0:128, 0:400],
                                                    m4z, m4z, ADD, SUB)
                        else:
                            zt = ytmp_pool.tile([128, 400], f32, tag="zt",
                                                name="zt")
                            nc.scalar.activation(zt[0:128, 0:400],
                                                 p2s[1][0:128, 0:400],
                                                 COPY, bias=m4z, scale=1.0)
                            nc.gpsimd.tensor_scalar(zo[0:128, 400:800],
                                                    zt[0:128, 0:400],
                                                    m4z, None, SUB)
                        nc.sync.dma_start(out_d.ap()[b, s], zo[0:128, :])

            g1, g2, q_ops = {}, {}, {}
            T = B_PER_CORE * SUPERS_PER_B

            # PE p-state warmup: ~8 dummy matmuls on garbage SBUF keep the
            # PE continuously busy through the startup DMA window so the
            # clock is fully ramped (needs 3us busy) at the first real
            # matmul. Garbage values never escape: the PSUM target is
            # overwritten by conv1's start=True accumulation.
            warm_s = cpool.tile([1, 16], bf, tag="warm_s", name="warm_s")
            warm_r = cpool.tile([1, W], bf, tag="warm_r", name="warm_r")
            nc.gpsimd.memset(warm_s[0:1, :], 0.0)
            nc.gpsimd.memset(warm_r[0:1, :], 0.0)
            warm = []
            p_w = p1_pool.tile([120, 512], f32, tag="p1a", name="p1a")
            for i in range(WNUM):
                mm = nc.tensor.matmul(p_w[0:1, 0:WN], warm_s[0:1, 0:1],
                                      warm_r[0:1, 0:WN], start=True, stop=True)
                warm.append((mm, mm))

            for it in range(T + D1 + D2):
                if it < T:
                    load(it)
                if it == 0:
                    # after the first x load so the two startup DMA chains
                    # (input prefetch, stationaries) overlap on HWDGE; conv1's
                    # matrices (first S1_END cols) ship first so the first
                    # conv1 isn't gated on the full stationary transfer
                    S1_END = S_OFF["s1e_0"]
                    nc.sync.dma_start(smat_t[0:121, 0:S1_END],
                                      smat_d.ap()[:, 0:S1_END])
                if it == SMAT2_IT:
                    # the bulk of the stationaries (conv2/edge matrices,
                    # first needed ~6us in) ships after the early x loads so
                    # their quants don't stall the DVE ahead of y-quants
                    S1_END = S_OFF["s1e_0"]
                    nc.sync.dma_start(smat_t[0:121, S1_END:S_TOT],
                                      smat_d.ap()[:, S1_END:S_TOT])
                if it >= D1 + D2:
                    conv2(it - D1 - D2)
                if D1 <= it < T + D1:
                    conv1(it - D1)

            # Force the PE stream into the gap-free interleave
            #   c1(s)h0, c1(s)h1, c2(s-1)h1, c2(s)h0, c1(s+1)h0, ...
            # so every conv2 group runs >=3 group-slots after the conv1
            # group whose y-quant feeds it. An edge super's single c2
            # group is carried past the next super's first c1 group to
            # keep the same distance.
            seq = list(warm)
            carry = None
            for s in range(T):
                gs = list(g1[s])
                seq.append(gs[0])
                if carry is not None:
                    seq.append(carry)
                    carry = None
                seq.extend(gs[1:])
                if s >= 1 and len(g2[s - 1]) > 1:
                    seq.append(g2[s - 1][1])
                if len(g2[s]) == 1:
                    carry = g2[s][0]
                else:
                    seq.append(g2[s][0])
            if len(g2[T - 1]) > 1:
                seq.append(g2[T - 1][1])
            if carry is not None:
                seq.append(carry)
            if SWP0:
                # startup prime: run c1(1)h0 before c2(0)h0 so the first
                # y-quant's latency bubble is absorbed by useful conv1 work
                i0 = WNUM + 2
                if seq[i0] is g2[0][0] and seq[i0 + 1] is g1[1][0]:
                    seq[i0], seq[i0 + 1] = seq[i0 + 1], seq[i0]
            if SWPE:
                # tail prime: pull the final edge's conv1 three slots earlier
                # so its y-quant is ready before the last conv2 groups
                c1e = g1[T - 1][0]
                i1 = seq.index(c1e)
                if i1 >= 3:
                    seq.pop(i1)
                    seq.insert(i1 - 3, c1e)
            for (pf, pl), (nf, nl) in zip(seq, seq[1:]):
                add_dep_helper(nf.ins, pl.ins, sync=False, reason="pe order")

    nc.compile()
    return nc


def _get_prog(scal_key, scal):
    if scal_key not in _prog_cache:
        _prog_cache[scal_key] = build_program(scal)
    return _prog_cache[scal_key]


def make_in_maps(x, consts, scal):
    smat = pack_smat(consts)
    return [{"x": x[c * B_PER_CORE:(c + 1) * B_PER_CORE], "smat": smat}
            for c in range(N_CORES)]


def kernel(x, w1, b1, w2, b2, s_in, s_w1, s_o1, s_w2, s_o2):
    x = np.ascontiguousarray(np.asarray(x, dtype=np.float32))
    assert x.shape == (32, CIN, H, W)
    # clamp-free fast path requires |x| comfortably inside the int8 range
    assert np.abs(x).max() < 7.9, "x clamp would be active; exact path invalid"
    consts, scal = _make_consts(np.asarray(w1), np.asarray(b1), np.asarray(w2),
                                np.asarray(b2), s_in, s_w1, s_o1, s_w2, s_o2)
    scal_key = tuple(sorted((k, float(v)) for k, v in scal.items()))
    nc = _get_prog(scal_key, scal)
    in_maps = make_in_maps(x, consts, scal)
    res = bass_utils.run_bass_kernel_spmd(nc, in_maps, core_ids=list(range(N_CORES)))

    out = np.concatenate(
        [unpack_core(np.asarray(res.results[c]["out"])) for c in range(N_CORES)],
        axis=0)
    return out


def unpack_core(a):
    """[B, super, wm, 800=(h,wt,co,ol)] bf16 -> [B, CO, 508, 508] f32."""
    a = np.asarray(a).astype(np.float32)
    main = a[:, :SUPERS_PER_B - 1].reshape(
        B_PER_CORE, SUPERS_PER_B - 1, 128, 2, NWT, COUT, 10)
    # [b, s, wm, h, wt, co, ol] -> z[b, co, 20s+10h+ol, 128wt+wm]
    main = np.transpose(main, (0, 5, 1, 3, 6, 4, 2)).reshape(
        B_PER_CORE, COUT, 500, 512)
    edge = a[:, SUPERS_PER_B - 1, :, :320].reshape(
        B_PER_CORE, 128, NWT, COUT, 8)
    # [b, wm, wt, co, ol] -> z[b, co, 500+ol, 128wt+wm]
    edge = np.transpose(edge, (0, 3, 4, 2, 1)).reshape(
        B_PER_CORE, COUT, 8, 512)
    z = np.concatenate([main, edge], axis=2)
    return z[:, :, :H2, :W2]

